# Initial kernel scaffold
#
"""Trainium2 Bass kernel for the blob-layer problem.

Computes out[b, c] = sum_hw x[b, hw] * curves[hw, c] / (H*W) where
curves[hw, c] = clip(factor_c * exp(-((xs-px_c)^2 + (ys-py_c)^2)/s2_c) * w_c).

Strategy (8 NeuronCores, SPMD):
- Shard the pixel (contraction) dim into 8 y-bands of 28 rows; each core
  computes a partial (B, C) output which the host sums.
- Per core, prune curve columns whose blob cannot reach its band
  (|py - band| > sqrt(T*s2)); contributions beyond that are < e^-T ~ 1e-11.
- grid is a rank-4 bilinear form:
    -grid = 2*px*xs + 2*py*ys - (px^2+py^2) - (xs^2+ys^2)
  so a K=4 fp32 matmul produces G = -grid for a 128-pixel tile against all
  kept columns. A DVE multiply by a replicated 1/s2 row gives M = -grid/s2
  (exact fp32; reduced-precision matmul is NOT usable here because 1/s2
  amplifies absolute error up to 1000x). ScalarE Exp produces e in bf16,
  and bf16 matmuls accumulate out[b, c] += x[hw, b] * e[hw, c] in PSUM.
- The clip never binds when max|factor*w| <= CAP (e <= 1), which holds for
  these inputs, so factor*w/npix is folded into a final per-column scale.
"""
import os
import sys

sys.path.insert(0, "/opt/trn_rl_repo")

import ml_dtypes
import numpy as np

import concourse.bass as bass
import concourse.tile as tile
from concourse import mybir
from concourse.bass_utils import run_bass_kernel_spmd

H, W, B, C = 224, 224, 256, 1024
NDEV = 8
ROWS = H // NDEV          # 28 rows per band
HWD = ROWS * W            # 6272 pixels per band
NT = HWD // 128           # 49 pixel tiles per band
EPS = 0.001
CAP = 2000.0
NPIX = float(H * W)
T_PRUNE = 25.0            # exp(-25) ~ 1.4e-11: dropped-column contribution bound

last_results = None       # BassKernelResults of the most recent run (for profiling)


def _build_program(nc_cols):
    """Emit the SPMD Bass program for NC kept/padded columns per core."""
    NC = nc_cols
    nc = bass.Bass()
    f32 = mybir.dt.float32
    bf16 = mybir.dt.bfloat16

    d_xT = nc.declare_dram_parameter("xT", [NT, 128, B], bf16, isOutput=False)
    d_Bm = nc.declare_dram_parameter("Bm", [4, HWD], f32, isOutput=False)
    d_Am = nc.declare_dram_parameter("Am", [4, NC], f32, isOutput=False)
    d_Rm = nc.declare_dram_parameter("Rm", [128, NC], f32, isOutput=False)
    d_Fw = nc.declare_dram_parameter("Fw", [128, NC], f32, isOutput=False)
    d_out = nc.declare_dram_parameter("out", [2, 128, NC], f32, isOutput=True)

    c_chunks = [(c0, min(512, NC - c0)) for c0 in range(0, NC, 512)]

    with tile.TileContext(nc) as tc:
        with (
            tc.tile_pool(name="const", bufs=1) as cpool,
            tc.tile_pool(name="xp", bufs=3) as xp,
            tc.tile_pool(name="mp", bufs=2) as mp,
            tc.tile_pool(name="ep", bufs=3) as ep,
            tc.tile_pool(name="op", bufs=1) as op,
            tc.tile_pool(name="psG", bufs=2, space="PSUM") as psG,
            tc.tile_pool(name="psO", bufs=1, space="PSUM") as psO,
        ):
            Bm = cpool.tile([4, HWD], f32, tag="Bm")
            Am = cpool.tile([4, NC], f32, tag="Am")
            Rm = cpool.tile([128, NC], f32, tag="Rm")
            Fw = cpool.tile([128, NC], f32, tag="Fw")
            nc.sync.dma_start(Bm[:], d_Bm[:])
            nc.sync.dma_start(Am[:], d_Am[:])
            nc.sync.dma_start(Rm[:], d_Rm[:])
            nc.sync.dma_start(Fw[:], d_Fw[:])

            Op0 = psO.tile([128, NC], f32, tag="op0")
            Op1 = psO.tile([128, NC], f32, tag="op1")

            def emit_main(j, e, xt):
                first, last = j == 0, j == NT - 1
                for bb, Opx in ((0, Op0), (1, Op1)):
                    for c0, cw in c_chunks:
                        nc.tensor.matmul(
                            Opx[:, c0 : c0 + cw],
                            xt[:, bb * 128 : (bb + 1) * 128],
                            e[:, c0 : c0 + cw],
                            start=first,
                            stop=last,
                            skip_group_check=True,
                        )

            pending = []
            for t in range(NT):
                xt = xp.tile([128, B], bf16, tag="xt")
                nc.sync.dma_start(xt[:], d_xT[t])

                Gp = psG.tile([128, NC], f32, tag="Gp")
                for c0, cw in c_chunks:
                    nc.tensor.matmul(
                        Gp[:, c0 : c0 + cw],
                        Bm[:, t * 128 : (t + 1) * 128],
                        Am[:, c0 : c0 + cw],
                        start=True,
                        stop=True,
                        skip_group_check=True,
                    )
                Msb = mp.tile([128, NC], f32, tag="M")
                nc.vector.tensor_mul(Msb[:], Gp[:], Rm[:])
                e = ep.tile([128, NC], bf16, tag="e")
                nc.scalar.activation(e[:], Msb[:], mybir.ActivationFunctionType.Exp)

                pending.append((t, e, xt))
                if len(pending) > 1:
                    emit_main(*pending.pop(0))
            while pending:
                emit_main(*pending.pop(0))

            out_sb = op.tile([128, 2 * NC], f32, tag="out")
            nc.vector.tensor_mul(out_sb[:, 0:NC], Op0[:], Fw[:])
            nc.vector.tensor_mul(out_sb[:, NC : 2 * NC], Op1[:], Fw[:])
            nc.sync.dma_start(d_out[0], out_sb[:, 0:NC])
            nc.sync.dma_start(d_out[1], out_sb[:, NC : 2 * NC])

    return nc


def kernel(x, positions, sigmas, curve_weights, xs, ys):
    global last_results
    x = np.asarray(x, dtype=np.float32)
    px = np.asarray(positions, dtype=np.float64)[0, 0, :, 1]
    py = np.asarray(positions, dtype=np.float64)[0, 0, :, 0]
    sg = np.asarray(sigmas, dtype=np.float64)[0, 0]
    w = np.asarray(curve_weights, dtype=np.float64)[0, 0]
    xs = np.asarray(xs, dtype=np.float64)
    ys = np.asarray(ys, dtype=np.float64)

    s2 = 2.0 * sg * sg + EPS
    factor = 1.0 / (2.0 * np.pi * sg * sg + EPS)
    fw = factor * w
    # clip(curves) is identity when max|factor*w| <= CAP since exp(...) <= 1
    assert np.abs(fw).max() <= CAP, "clip binds; folded-scale scheme invalid"

    r = np.sqrt(T_PRUNE * s2)
    keep_idx = []
    for d in range(NDEV):
        h0 = d * ROWS
        y0, y1 = ys[h0, 0], ys[h0 + ROWS - 1, 0]
        keep_idx.append(np.where((py >= y0 - r) & (py <= y1 + r))[0])
    NC = max(128, -(-max(len(i) for i in keep_idx) // 128) * 128)
    assert NC <= 1024

    in_maps = []
    for d in range(NDEV):
        h0 = d * ROWS
        rows = slice(h0, h0 + ROWS)
        xs_b = xs[rows].ravel()
        ys_b = ys[rows].ravel()
        Bm = np.stack(
            [xs_b, ys_b, np.ones(HWD), xs_b * xs_b + ys_b * ys_b]
        ).astype(np.float32)

        idx = keep_idx[d]
        nk = len(idx)
        Am = np.zeros((4, NC), np.float32)
        Am[0, :nk] = 2.0 * px[idx]
        Am[1, :nk] = 2.0 * py[idx]
        Am[2, :nk] = -(px[idx] ** 2 + py[idx] ** 2)
        Am[3, :] = -1.0
        R = np.ones(NC, np.float64)
        R[:nk] = 1.0 / s2[idx]
        F = np.zeros(NC, np.float64)
        F[:nk] = fw[idx] / NPIX

        xT = np.ascontiguousarray(
            x[:, rows, :].reshape(B, HWD).T
        ).reshape(NT, 128, B).astype(ml_dtypes.bfloat16)

        in_maps.append(
            {
                "xT": xT,
                "Bm": Bm,
                "Am": Am,
                "Rm": np.ascontiguousarray(
                    np.broadcast_to(R.astype(np.float32), (128, NC))
                ),
                "Fw": np.ascontiguousarray(
                    np.broadcast_to(F.astype(np.float32), (128, NC))
                ),
            }
        )

    nc = _build_program(NC)
    trace = bool(os.environ.get("BLOB_TRACE"))
    last_results = run_bass_kernel_spmd(
        nc, in_maps, list(range(NDEV)), trace=trace
    )

    out = np.zeros((B, C), np.float32)
    for d in range(NDEV):
        nk = len(keep_idx[d])
        dev = np.asarray(last_results.results[d]["out"], np.float32).reshape(B, NC)
        out[:, keep_idx[d]] += dev[:, :nk]
    return out


# revision 23
# speedup vs baseline: 2.9501x; 2.9501x over previous
"""Trainium2 Bass kernel for the blob-layer problem.

Computes out[b, c] = sum_hw x[b, hw] * curves[hw, c] / (H*W) where
curves[hw, c] = clip(factor_c * exp(-((xs-px_c)^2 + (ys-py_c)^2)/s2_c) * w_c).

Strategy (8 NeuronCores, SPMD):
- Shard the pixel (contraction) dim into 8 y-bands of 28 rows; each core
  computes a partial (B, C) output which the host sums.
- Per core, prune curve columns whose blob cannot reach its band
  (|py - band| > sqrt(T*s2)); contributions beyond that are < e^-T ~ 1e-11.
- grid is a rank-4 bilinear form:
    -grid = 2*px*xs + 2*py*ys - (px^2+py^2) - (xs^2+ys^2)
  so a K=4 fp32 matmul produces G = -grid for a 128-pixel tile against all
  kept columns. A DVE multiply by a replicated 1/s2 row gives M = -grid/s2
  (exact fp32; reduced-precision matmul is NOT usable here because 1/s2
  amplifies absolute error up to 1000x). ScalarE Exp produces e in bf16,
  and bf16 matmuls accumulate out[b, c] += x[hw, b] * e[hw, c] in PSUM.
- The clip never binds when max|factor*w| <= CAP (e <= 1), which holds for
  these inputs, so factor*w/npix is folded into a final per-column scale.
"""
import os
import sys

sys.path.insert(0, "/opt/trn_rl_repo")

import ml_dtypes
import numpy as np

import concourse.bass as bass
import concourse.bacc as bacc
import concourse.tile as tile
from concourse import mybir
from concourse.bass_utils import run_bass_kernel_spmd

H, W, B, C = 224, 224, 256, 1024
NDEV = 8
ROWS = H // NDEV          # 28 rows per band
HWD = ROWS * W            # 6272 pixels per band
NT = HWD // 128           # 49 pixel tiles per band
EPS = 0.001
CAP = 2000.0
NPIX = float(H * W)
T_PRUNE = 25.0            # exp(-25) ~ 1.4e-11: dropped-column contribution bound

last_results = None       # BassKernelResults of the most recent run (for profiling)


def _build_program(nc_cols, reps=1):
    """Emit the SPMD Bass program for NC kept/padded columns per core.

    Sync-wait discipline: a fused fp32 LDWEIGHTS+MATMUL accepts only ONE
    semaphore wait, so every fp32 matmul may depend on at most one new tick.
    The A matrix is pre-scaled by 1/s2 so the K=4 fp32 matmul yields
    M = -grid/s2 directly in PSUM and ScalarE's Exp reads PSUM — no DVE
    stage. The G matmul's only dep is then a single PE sem value (PSUM slot
    release merged with the weight-register WAR); its ACT release is already
    observed via the preceding main matmul's e-wait. bf16 main matmuls get a
    split LDWEIGHTS, so their x-DMA wait and e-ACT wait land on separate
    instructions. Bm/Am share one DMA so the first G matmul sees one queue
    sem.
    """
    NC = nc_cols
    nc = bacc.Bacc()
    f32 = mybir.dt.float32
    bf16 = mybir.dt.bfloat16

    d_xT = nc.declare_dram_parameter("xT", [NT, 128, B], bf16, isOutput=False)
    d_BA = nc.declare_dram_parameter("BA", [4, HWD + NC], f32, isOutput=False)
    d_Fw = nc.declare_dram_parameter("Fw", [128, NC], f32, isOutput=False)
    d_out = nc.declare_dram_parameter("out", [2, 128, NC], f32, isOutput=True)

    c_chunks = [(c0, min(512, NC - c0)) for c0 in range(0, NC, 512)]

    with tile.TileContext(nc) as tc:
        with (
            tc.tile_pool(name="const", bufs=1) as cpool,
            tc.tile_pool(name="ep", bufs=3) as ep,
            tc.tile_pool(name="op", bufs=1) as op,
            tc.tile_pool(name="psG", bufs=2, space="PSUM") as psG,
            tc.tile_pool(name="psO", bufs=1, space="PSUM") as psO,
        ):
            BA = cpool.tile([4, HWD + NC], f32, tag="BA")
            Fw = cpool.tile([128, NC], f32, tag="Fw")
            nc.gpsimd.dma_start(BA[:], d_BA[:])
            nc.gpsimd.dma_start(Fw[:], d_Fw[:])

            # whole x band stays SBUF-resident (25KB/partition): a few large
            # DMAs write disjoint ranges of one tile, so no slot-recycle or
            # queue-ring waits exist and each main LDWEIGHTS waits on at most
            # one DMA queue sem.
            xfull = cpool.tile([128, NT * B], bf16, tag="xfull")
            grp = (NT + 6) // 7
            for t0 in range(0, NT, grp):
                t1 = min(t0 + grp, NT)
                nc.sync.dma_start(
                    xfull[:, t0 * B : t1 * B].rearrange(
                        "p (t b) -> p t b", t=t1 - t0
                    ),
                    d_xT[t0:t1].rearrange("t p b -> p t b"),
                )

            Op0 = psO.tile([128, NC], f32, tag="op0")
            Op1 = psO.tile([128, NC], f32, tag="op1")

            def emit_main(j, e):
                first, last = j == 0, j == NT - 1
                for bb, Opx in ((0, Op0), (1, Op1)):
                    for c0, cw in c_chunks:
                        nc.tensor.matmul(
                            Opx[:, c0 : c0 + cw],
                            xfull[:, j * B + bb * 128 : j * B + (bb + 1) * 128],
                            e[:, c0 : c0 + cw],
                            start=first,
                            stop=last,
                            skip_group_check=True,
                        )

            # reps>1 repeats the identical computation (timing harness only;
            # each rep's start=True resets the accumulators, so the final
            # output is unchanged).
            for _ in range(reps):
                pending = []
                for t in range(NT):
                    Gp = psG.tile([128, NC], f32, tag="Gp")
                    for c0, cw in c_chunks:
                        nc.tensor.matmul(
                            Gp[:, c0 : c0 + cw],
                            BA[:, t * 128 : (t + 1) * 128],
                            BA[:, HWD + c0 : HWD + c0 + cw],
                            start=True,
                            stop=True,
                            skip_group_check=True,
                        )
                    e = ep.tile([128, NC], bf16, tag="e")
                    nc.scalar.activation(e[:], Gp[:], mybir.ActivationFunctionType.Exp)

                    pending.append((t, e))
                    if len(pending) > 1:
                        emit_main(*pending.pop(0))
                while pending:
                    emit_main(*pending.pop(0))

            out_sb = op.tile([128, 2 * NC], f32, tag="out")
            nc.vector.tensor_mul(out_sb[:, 0:NC], Op0[:], Fw[:])
            nc.vector.tensor_mul(out_sb[:, NC : 2 * NC], Op1[:], Fw[:])
            nc.sync.dma_start(d_out[0], out_sb[:, 0:NC])
            nc.sync.dma_start(d_out[1], out_sb[:, NC : 2 * NC])

    nc.compile()
    return nc


def _prepare(x, positions, sigmas, curve_weights, xs, ys):
    x = np.asarray(x, dtype=np.float32)
    px = np.asarray(positions, dtype=np.float64)[0, 0, :, 1]
    py = np.asarray(positions, dtype=np.float64)[0, 0, :, 0]
    sg = np.asarray(sigmas, dtype=np.float64)[0, 0]
    w = np.asarray(curve_weights, dtype=np.float64)[0, 0]
    xs = np.asarray(xs, dtype=np.float64)
    ys = np.asarray(ys, dtype=np.float64)

    s2 = 2.0 * sg * sg + EPS
    factor = 1.0 / (2.0 * np.pi * sg * sg + EPS)
    fw = factor * w
    # clip(curves) is identity when max|factor*w| <= CAP since exp(...) <= 1
    assert np.abs(fw).max() <= CAP, "clip binds; folded-scale scheme invalid"

    r = np.sqrt(T_PRUNE * s2)
    keep_idx = []
    for d in range(NDEV):
        h0 = d * ROWS
        y0, y1 = ys[h0, 0], ys[h0 + ROWS - 1, 0]
        keep_idx.append(np.where((py >= y0 - r) & (py <= y1 + r))[0])
    NC = max(128, -(-max(len(i) for i in keep_idx) // 128) * 128)
    assert NC <= 1024

    in_maps = []
    for d in range(NDEV):
        h0 = d * ROWS
        rows = slice(h0, h0 + ROWS)
        xs_b = xs[rows].ravel()
        ys_b = ys[rows].ravel()
        Bm = np.stack(
            [xs_b, ys_b, np.ones(HWD), xs_b * xs_b + ys_b * ys_b]
        ).astype(np.float32)

        idx = keep_idx[d]
        nk = len(idx)
        # A columns pre-scaled by 1/s2 so the matmul yields M = -grid/s2
        Am = np.zeros((4, NC), np.float32)
        Am[0, :nk] = 2.0 * px[idx] / s2[idx]
        Am[1, :nk] = 2.0 * py[idx] / s2[idx]
        Am[2, :nk] = -(px[idx] ** 2 + py[idx] ** 2) / s2[idx]
        Am[3, :nk] = -1.0 / s2[idx]
        Am[3, nk:] = -1.0
        BAm = np.concatenate([Bm, Am], axis=1)
        F = np.zeros(NC, np.float64)
        F[:nk] = fw[idx] / NPIX

        xT = np.ascontiguousarray(
            x[:, rows, :].reshape(B, HWD).T
        ).reshape(NT, 128, B).astype(ml_dtypes.bfloat16)

        in_maps.append(
            {
                "xT": xT,
                "BA": BAm,
                "Fw": np.ascontiguousarray(
                    np.broadcast_to(F.astype(np.float32), (128, NC))
                ),
            }
        )
    return NC, in_maps, keep_idx


def _gather(results, keep_idx, NC):
    out = np.zeros((B, C), np.float32)
    for d in range(NDEV):
        nk = len(keep_idx[d])
        dev = np.asarray(results[d]["out"], np.float32).reshape(B, NC)
        out[:, keep_idx[d]] += dev[:, :nk]
    return out


def kernel(x, positions, sigmas, curve_weights, xs, ys):
    global last_results
    NC, in_maps, keep_idx = _prepare(x, positions, sigmas, curve_weights, xs, ys)
    nc = _build_program(NC)
    trace = bool(os.environ.get("BLOB_TRACE"))
    last_results = run_bass_kernel_spmd(
        nc, in_maps, list(range(NDEV)), trace=trace
    )
    return _gather(last_results.results, keep_idx, NC)
